# revision 14
# baseline (speedup 1.0000x reference)
"""AdaptiveAttentionLayer on 8 TRN2 NeuronCores.

Full inputs in, full output out. Sharding: data-parallel over batch (B=4)
x 2-way sequence-parallel over the 4096 query rows -> 8 cores, each core
computes a [2048, 256] slice of one batch item's output.

The PE streams moving data at ~1 row/cycle regardless of dtype, so the
only matmul lever is fewer rows: the attention core (scores, A@V,
A@V^2 - 87% of PE work) runs as fp8e4 DoubleRow matmuls, which pack two
128-deep contractions per pass (2x). K^T is pre-normalized (1/||k||
folded in) so the softmax exp needs no per-key scale and one fused Exp
covers a 2-bank PSUM score pair. Softmax denominators: GpSimd sums each
fp8 P pair into fp16, DVE accumulates fp16 at its 4x perf mode. All
sqrt/rsqrt/reciprocal are Ln+Exp compositions so the scalar engine
keeps ONE activation table loaded (ln/exp/square/copy). PSUM plan:
score-pair 2 banks + M/E2 accumulators 4 + broadcast 1 + small rows 1.

Per-core device pipeline (channel-major / transposed layouts):
  - instance-norm stats of content/style (free-axis reductions)
  - V = style @ Wv   row-major; bias broadcast-added; V2=V*V (fp8 pairs)
  - K^T = (diag(inv_s) Wk)^T style^T + bias  channel-major bf16,
    column-l2-normalized via PE colsums + Ln/Exp + PE broadcast -> fp8
  - Q^T = Wq^T norm_content^T, l2norm likewise -> fp8
  - scores^T pair = Khat_pair qhat (fp8 DoubleRow, 512-query chunks)
  - P = exp(scores) fused per pair -> fp8
  - M^T = V^T P^T, E2^T = (V*V)^T P^T (fp8 DoubleRow, PSUM-accumulated)
  - r = sum_k P (GpSimd pair adds + DVE fp16 + PE ones-matmul),
    out = sqrt(relu(E2/r-(M/r)^2)) * norm_content + M/r
"""

import sys

if "/opt/trn_rl_repo" not in sys.path:
    sys.path.insert(0, "/opt/trn_rl_repo")

import os
import numpy as np
import ml_dtypes

import concourse.bass as bass
import concourse.mybir as mybir
import concourse.tile as tile
from concourse.bass_utils import run_bass_kernel_spmd

F32 = mybir.dt.float32
BF16 = mybir.dt.bfloat16
F16 = mybir.dt.float16
FP8 = mybir.dt.float8e4
PM = mybir.MatmulPerfMode
ALU = mybir.AluOpType
ACTF = mybir.ActivationFunctionType

B, H, W, C = 4, 64, 64, 256
N = H * W          # 4096 key/query rows per batch item
QH = N // 2        # 2048 query rows per core
NK = N // 128      # 32 key tiles
NPR = NK // 2      # 16 key-tile pairs (fp8 DoubleRow)
QC = 512           # query chunk (matmul moving free dim)
NQC = QH // QC     # 4 query chunks per core
EPS_IN = 1e-5      # instance norm eps
EPS_L2 = 1e-12     # l2norm eps
EPS_LN = 1e-30     # guards Ln(0) in sqrt-by-Ln/Exp

LAST_EXEC_NS = {"v": None}

NPBF16 = ml_dtypes.bfloat16
NPFP8 = mybir.dt.np(FP8)


def _pack_pairs(a):
    """[256, F] -> [128, 2*F] fp8 pair layout (dim1 = which 128-half)."""
    f = a.shape[1]
    return np.ascontiguousarray(
        a.reshape(2, 128, f).transpose(1, 0, 2).reshape(128, 2 * f)
    ).astype(NPFP8)


def _legalize_waits(nc):
    """This walrus build accepts at most ONE sync wait per instruction
    ('Too many sync wait commands'). Hoist extra waits onto same-engine
    NOPs inserted immediately before the offending instruction."""
    fn = nc.m.functions[0]
    nfix = 0
    for bb in fn.blocks:
        i = 0
        while i < len(bb.instructions):
            inst = bb.instructions[i]
            si = inst.sync_info
            if si is not None and len(si.on_wait) > 1:
                waits = list(si.on_wait)
                for j, w in enumerate(waits[:-1]):
                    nop = mybir.InstNoOp(
                        name=nc.get_next_instruction_name(), ins=[], outs=[]
                    )
                    nop.engine = inst.engine
                    nop.sync_info = mybir.SyncInfo(on_wait=[w], on_update=[])
                    nc.register_instruction(nop)
                    bb.instructions.insert(i + j, nop)
                i += len(waits) - 1
                inst.sync_info = mybir.SyncInfo(
                    on_wait=[waits[-1]], on_update=list(si.on_update)
                )
                nfix += 1
            i += 1
    return nfix


def _install_profshim():
    """antenv.axon_hooks is absent in this image; provide it (ctypes into
    libaxon_pjrt.so) plus an offline-safe upload_artifacts so trace=True
    yields exec_time_ns."""
    import contextlib, ctypes, types

    if "antenv.axon_hooks" in sys.modules:
        return
    so = "/opt/axon/libaxon_pjrt.so"
    hook = None
    if os.path.exists(so):
        lib = ctypes.CDLL(so)
        if hasattr(lib, "axon_start_nrt_profile"):
            lib.axon_start_nrt_profile.argtypes = [
                ctypes.POINTER(ctypes.c_int64),
                ctypes.c_size_t,
            ]
            lib.axon_start_nrt_profile.restype = ctypes.c_int64
            lib.axon_stop_nrt_profile.argtypes = [ctypes.c_char_p]
            lib.axon_stop_nrt_profile.restype = ctypes.c_int64

            @contextlib.contextmanager
            def _hook(output_dir, device_ids):
                import jax

                jax.devices()
                if device_ids:
                    ids = (ctypes.c_int64 * len(device_ids))(*device_ids)
                    rc = lib.axon_start_nrt_profile(ids, len(device_ids))
                else:
                    rc = lib.axon_start_nrt_profile(None, 0)
                if rc != 0:
                    raise RuntimeError(f"axon_start_nrt_profile rc={rc}")
                try:
                    yield
                finally:
                    n = lib.axon_stop_nrt_profile(str(output_dir).encode())
                    print(f"profile: {n} ntff file(s) -> {output_dir}",
                          file=sys.stderr)

            hook = _hook

    mod = types.ModuleType("antenv.axon_hooks")
    mod.get_axon_ntff_profile_hook = lambda: hook
    mod.set_axon_ntff_profile_hook = lambda h: None
    sys.modules["antenv.axon_hooks"] = mod

    import concourse.bass_utils as bu

    bu.upload_artifacts = lambda tmpdir: tmpdir


def build_nc():
    nc = bass.Bass()

    xa_e = nc.declare_dram_parameter("xa", [C, QH], BF16, isOutput=False)
    xb_e = nc.declare_dram_parameter("xb", [C, QH], BF16, isOutput=False)
    st_e = nc.declare_dram_parameter("st", [128, 2 * N], FP8, isOutput=False)
    wq_e = nc.declare_dram_parameter("wq", [128, 2 * C], FP8, isOutput=False)
    wk_e = nc.declare_dram_parameter("wk", [C, C], BF16, isOutput=False)
    wv_e = nc.declare_dram_parameter("wv", [128, 2 * C], FP8, isOutput=False)
    bqr_e = nc.declare_dram_parameter("bqr", [C, 1], F32, isOutput=False)
    bkr_e = nc.declare_dram_parameter("bkr", [C, 1], F32, isOutput=False)
    bvr_e = nc.declare_dram_parameter("bvr", [1, C], BF16, isOutput=False)
    out_e = nc.declare_dram_parameter("out", [C, QH], F32, isOutput=True)

    NCH_K = N // QC       # 8 key chunks
    DCH = 1024
    SCH = 2048            # stats chunk

    with tile.TileContext(nc) as tc, \
            nc.allow_low_precision(reason="fp8/bf16 attention pipeline"):
        with tc.tile_pool(name="persist", bufs=1) as pp:
            ones_col = pp.tile([128, 1], BF16)  # colsum stationary
            ones_c16 = pp.tile([128, 1], F16)   # denom colsum stationary
            ones_rbf = pp.tile([1, 128], BF16)  # bv broadcast stationary
            ones_r16 = pp.tile([1, 128], F16)   # rinv/iqr/invk broadcast
            eps_in_t = pp.tile([128, 1], F32)
            eps_l2_t = pp.tile([128, 1], F32)
            eps_ln_t = pp.tile([128, 1], F32)
            wq8 = pp.tile([128, 2, C], FP8)
            wk_s = [pp.tile([128, C], BF16, name=f"wk{i}") for i in range(2)]
            wk8 = pp.tile([128, 2, C], FP8)
            wv8 = pp.tile([128, 2, C], FP8)
            nct8 = pp.tile([128, 2, QH], FP8)
            bqc = [pp.tile([128, 1], F32, name=f"bqc{i}") for i in range(2)]
            bkc = [pp.tile([128, 1], F32, name=f"bkc{i}") for i in range(2)]
            bkc_f = [pp.tile([128, 1], F32, name=f"bkf{i}") for i in range(2)]
            bv_row = pp.tile([1, C], BF16)
            bvb = pp.tile([128, C], F32)
            # DoubleRow pair layouts (dim1 = which half of the 256-deep
            # contraction):
            #   knt8[:, co, k]      Khat^T chans co*128.., key k
            #   qnt8[:, co, q]      Qhat^T chans co*128..
            #   v8[:, pr, w, c]     V[key tile 2pr+w, chan c]
            knt_bf = pp.tile([128, 2, N], BF16)
            knt8 = pp.tile([128, 2, N], FP8)
            qnt8 = pp.tile([128, 2, QH], FP8)
            qnt = [pp.tile([128, QH], BF16, name=f"qnt{i}") for i in range(2)]
            nct = [pp.tile([128, QH], BF16, name=f"nct{i}") for i in range(2)]
            v8 = pp.tile([128, NPR, 2, C], FP8)
            v28 = pp.tile([128, NPR, 2, C], FP8)
            mean_s = [pp.tile([128, 1], F32, name=f"ms{i}") for i in range(2)]
            inv_s = [pp.tile([128, 1], F32, name=f"is{i}") for i in range(2)]
            mean_x = [pp.tile([128, 1], F32, name=f"mx{i}") for i in range(2)]
            inv_x = [pp.tile([128, 1], F32, name=f"ix{i}") for i in range(2)]

            nc.vector.memset(ones_col[:], 1.0)
            nc.vector.memset(ones_c16[:], 1.0)
            nc.vector.memset(ones_rbf[:], 1.0)
            nc.vector.memset(ones_r16[:], 1.0)
            nc.vector.memset(eps_in_t[:], EPS_IN)
            nc.vector.memset(eps_l2_t[:], EPS_L2)
            nc.vector.memset(eps_ln_t[:], EPS_LN)

            # ================= phase 1: stats + projections =================
            with (
                tc.tile_pool(name="inputs", bufs=1) as tp,
                tc.tile_pool(name="w1", bufs=2) as w1,
                tc.tile_pool(name="psum1", bufs=3, space="PSUM") as ps1,
            ):
                st8 = tp.tile([128, 2, N], FP8, name="st8")
                xa_t = [tp.tile([128, QH], BF16, name=f"xa{i}")
                        for i in range(2)]
                for w in range(2):
                    nc.sync.dma_start(wv8[:, w, :], wv_e[:, w * C:(w + 1) * C])
                    nc.sync.dma_start(wq8[:, w, :], wq_e[:, w * C:(w + 1) * C])
                for i in range(2):
                    nc.sync.dma_start(wk_s[i][:],
                                      wk_e[i * 128:(i + 1) * 128, :])
                    nc.sync.dma_start(bqc[i][:], bqr_e[i * 128:(i + 1) * 128, :])
                    nc.sync.dma_start(bkc[i][:], bkr_e[i * 128:(i + 1) * 128, :])
                nc.sync.dma_start(bv_row[:], bvr_e[:])
                for j in range(0, N, DCH):
                    for i in range(2):
                        nc.sync.dma_start(
                            st8[:, i, j:j + DCH],
                            st_e[:, i * N + j:i * N + j + DCH])
                for j in range(0, QH, DCH):
                    for i in range(2):
                        nc.sync.dma_start(
                            xa_t[i][:, j:j + DCH],
                            xa_e[i * 128:(i + 1) * 128, j:j + DCH],
                        )

                # bv broadcast for V row-major bias add
                ps_bc = ps1.tile([128, C], F32, name="ps_bc", tag="prj")
                nc.tensor.matmul(ps_bc[:], ones_rbf[:], bv_row[:])
                nc.vector.tensor_copy(bvb[:], ps_bc[:])

                def stats_closures(chunks, mean, inv, i):
                    """Return a list of closures; call them in order, spaced
                    between PE-heavy work. Last closure finalizes stats."""
                    nck = len(chunks)
                    parts = w1.tile([128, nck], F32, name="parts",
                                    bufs=2)
                    parts_m = w1.tile([128, nck], F16, name="parts_m",
                                      bufs=2)
                    out = []

                    def chunk_op(j, ch):
                        def go():
                            scr = w1.tile([128, N], BF16, name="sqscr",
                                          bufs=2)
                            nc.scalar.activation(
                                scr[:, 0:ch.free_size()], ch, ACTF.Square,
                                accum_out=parts[:, j:j + 1],
                            )
                            nc.vector.tensor_reduce(
                                parts_m[:, j:j + 1], ch,
                                axis=mybir.AxisListType.X, op=ALU.add,
                            )
                        return go

                    for j, ch in enumerate(chunks):
                        out.append(chunk_op(j, ch))

                    def finalize():
                        ssq = w1.tile([128, 1], F32, name="ssq")
                        nc.vector.reduce_sum(ssq[:], parts[:, 0:nck],
                                             axis=mybir.AxisListType.X)
                        ssum = w1.tile([128, 1], F32, name="ssum")
                        nc.vector.reduce_sum(ssum[:], parts_m[:, 0:nck],
                                             axis=mybir.AxisListType.X)
                        nc.vector.tensor_scalar_mul(mean[i][:], ssum[:],
                                                    1.0 / N)
                        ex2 = w1.tile([128, 1], F32, name="ex2")
                        nc.vector.tensor_scalar_mul(ex2[:], ssq[:], 1.0 / N)
                        msq = w1.tile([128, 1], F32, name="msq")
                        nc.vector.tensor_mul(msq[:], mean[i][:], mean[i][:])
                        var = w1.tile([128, 1], F32, name="var")
                        nc.vector.tensor_sub(var[:], ex2[:], msq[:])
                        # 1/sqrt(var+eps) = Exp(-0.5*Ln(var+eps))
                        lnv = w1.tile([128, 1], F32, name="lnv")
                        nc.scalar.activation(lnv[:], var[:], ACTF.Ln,
                                             bias=eps_in_t[:])
                        nc.scalar.activation(inv[i][:], lnv[:], ACTF.Exp,
                                             scale=-0.5)
                    out.append(finalize)
                    return out

                for i in range(2):
                    for op in stats_closures([st8[:, i, :]],
                                             mean_s, inv_s, i):
                        op()

                # ---- V projection (row-major; bias added at evacuation
                # straight into the fp8 pair layout); V2 = V*V behind it.
                # style-stats ops interleaved so they don't head-of-line
                # block the V PSUM evacuations
                for kt in range(NK):
                    ksl = slice(kt * 128, (kt + 1) * 128)
                    ps_v = ps1.tile([128, C], F32, name="ps_v", tag="prj")
                    nc.tensor.matmul(ps_v[:], st8[:, :, ksl], wv8[:],
                                     start=True, stop=True,
                                     perf_mode=PM.DoubleRow)
                    vdst = v8[:, kt // 2, kt % 2, :]
                    nc.vector.tensor_add(vdst, ps_v[:], bvb[:])
                    nc.gpsimd.tensor_mul(v28[:, kt // 2, kt % 2, :],
                                         vdst, vdst)

                # ---- fold style instance norm into Wk; column bias corr
                for i in range(2):
                    nc.vector.tensor_scalar_mul(wk_s[i][:], wk_s[i][:],
                                                inv_s[i][:])
                mu_inv = [w1.tile([128, 1], BF16, name=f"mi{i}")
                          for i in range(2)]
                for i in range(2):
                    nc.vector.tensor_mul(mu_inv[i][:], mean_s[i][:],
                                         inv_s[i][:])
                for co in range(2):
                    ps_c = ps1.tile([128, 1], F32, name="ps_c", tag="pn",
                                    bufs=2)
                    csl = slice(co * 128, (co + 1) * 128)
                    nc.tensor.matmul(ps_c[:], wk_s[0][:, csl],
                                     mu_inv[0][:], start=True, stop=False)
                    nc.tensor.matmul(ps_c[:], wk_s[1][:, csl],
                                     mu_inv[1][:], start=False, stop=True)
                    nc.vector.tensor_sub(bkc_f[co][:], bkc[co][:], ps_c[:])
                for w in range(2):
                    nc.vector.tensor_copy(wk8[:, w, :], wk_s[w][:])

                # ---- K^T projection (channel-major bf16) + column
                # sumsq + per-chunk l2 normalization into fp8 (pipelined
                # so the norm chain hides under later chunks' matmuls)

                def proj_t(src, w_t, bias_c, nch):
                    def colsum(ch, sq):
                        csl = slice(ch * QC, (ch + 1) * QC)
                        ps_n = ps1.tile([1, QC], F32, name="ps_n", tag="pn",
                                        bufs=2)
                        nc.tensor.matmul(ps_n[:], ones_col[:],
                                         sq[0][:], start=True, stop=False)
                        nc.tensor.matmul(ps_n[:], ones_col[:],
                                         sq[1][:], start=False, stop=True)
                        lnk = w1.tile([1, QC], F32, name="lnk", bufs=2)
                        nc.scalar.activation(lnk[:], ps_n[:], ACTF.Ln,
                                             bias=eps_l2_t[0:1, :])
                        ivr = w1.tile([1, QC], F16, name="ivr", bufs=2)
                        nc.scalar.activation(ivr[:], lnk[:], ACTF.Exp,
                                             scale=-0.5)
                        ps_b = ps1.tile([128, QC], F32, name="ps_b",
                                        tag="pbig")
                        nc.tensor.matmul(ps_b[:], ones_r16[:], ivr[:])
                        for co in range(2):
                            nc.vector.tensor_mul(knt8[:, co, csl],
                                                 knt_bf[:, co, csl], ps_b[:])

                    pend = None
                    for ch in range(nch):
                        csl = slice(ch * QC, (ch + 1) * QC)
                        sq = []
                        for co in range(2):
                            wsl = slice(co * 128, (co + 1) * 128)
                            ps_p = ps1.tile([128, QC], F32, name="ps_p",
                                            tag="pbig")
                            nc.tensor.matmul(ps_p[:], w_t[:, :, wsl],
                                             src[:, :, csl],
                                             start=True, stop=True,
                                             perf_mode=PM.DoubleRow)
                            kdst = knt_bf[:, co, csl]
                            nc.vector.tensor_scalar(
                                out=kdst, in0=ps_p[:],
                                scalar1=bias_c[co][:], scalar2=None,
                                op0=ALU.add)
                            s = w1.tile([128, QC], BF16, name="sqc", bufs=3)
                            nc.scalar.activation(s[:], ps_p[:], ACTF.Square,
                                                 bias=bias_c[co][:])
                            sq.append(s)
                        if pend is not None:
                            colsum(*pend)
                        pend = (ch, sq)
                    colsum(*pend)

                # content stats + norm_content before the K
                # projection so the Q-side chain unblocks early
                xbch = {}
                for i in range(2):
                    cb = tp.tile([128, QH], BF16, name="xbs", bufs=2)
                    nc.sync.dma_start(cb[:], xb_e[i * 128:(i + 1) * 128, :])
                    xbch[i] = cb
                for i in range(2):
                    for op in stats_closures([xa_t[i][:], xbch[i][:]],
                                             mean_x, inv_x, i):
                        op()
                for i in range(2):
                    nc.vector.tensor_scalar(
                        out=nct[i][:], in0=xa_t[i][:],
                        scalar1=mean_x[i][:], scalar2=inv_x[i][:],
                        op0=ALU.subtract, op1=ALU.mult,
                    )
                    nc.vector.tensor_copy(nct8[:, i, :], nct[i][:])

                proj_t(st8, wk8, bkc_f, NCH_K)


            # ========== phase 2: attention ==========
            with (
                tc.tile_pool(name="w2", bufs=2) as w2,
                tc.tile_pool(name="psum_acc", bufs=1, space="PSUM") as psa,
                tc.tile_pool(name="psum_pair", bufs=1, space="PSUM") as ppx,
                tc.tile_pool(name="psum_small", bufs=1, space="PSUM") as psl,
            ):
                state = {}
                qstate = {}

                def qproj_a(qc):
                    """Project Q chunk qc into qnt (channel-major) and
                    square for column norms."""
                    csl = slice(qc * QC, (qc + 1) * QC)
                    pq = ppx.tile([128, 2 * QC], F32, name="pair")
                    for co in range(2):
                        wsl = slice(co * 128, (co + 1) * 128)
                        osl = slice(co * QC, (co + 1) * QC)
                        nc.tensor.matmul(pq[:, osl], wq8[:, :, wsl],
                                         nct8[:, :, csl],
                                         start=True, stop=True,
                                         perf_mode=PM.DoubleRow)
                    sq = []
                    for co in range(2):
                        osl = slice(co * QC, (co + 1) * QC)
                        nc.vector.tensor_scalar(
                            out=qnt[co][:, csl], in0=pq[:, osl],
                            scalar1=bqc[co][:], scalar2=None, op0=ALU.add)
                        s = w2.tile([128, QC], BF16, name="qsq", bufs=2)
                        nc.scalar.activation(s[:], qnt[co][:, csl],
                                             ACTF.Square)
                        sq.append(s)
                    qstate[qc] = sq

                def qproj_b(qc):
                    """Column sumsq -> 1/norm row for chunk qc."""
                    sq = qstate.pop(qc)
                    ps_n = psl.tile([1, QC], F32, name="ps_n", tag="small")
                    nc.tensor.matmul(ps_n[:], ones_col[:], sq[0][:],
                                     start=True, stop=False)
                    nc.tensor.matmul(ps_n[:], ones_col[:], sq[1][:],
                                     start=False, stop=True)
                    lnq = w2.tile([1, QC], F32, name="lnq", bufs=1)
                    nc.scalar.activation(lnq[:], ps_n[:], ACTF.Ln,
                                         bias=eps_l2_t[0:1, :])
                    iqr = w2.tile([1, QC], F16, name="invr", bufs=2)
                    nc.scalar.activation(iqr[:], lnq[:], ACTF.Exp,
                                         scale=-0.5)
                    qstate[qc] = iqr

                def qproj_c(qc):
                    """Broadcast 1/norm and scale Q chunk qc columns."""
                    iqr = qstate.pop(qc)
                    csl = slice(qc * QC, (qc + 1) * QC)
                    ps_b = psa.tile([128, QC], F32, name="qps_b",
                                    tag="ps_rb")
                    nc.tensor.matmul(ps_b[:], ones_r16[:], iqr[:])
                    for co in range(2):
                        nc.vector.tensor_mul(qnt[co][:, csl],
                                             qnt[co][:, csl], ps_b[:])

                def qproj_d(qc):
                    """fp8 copy of the scaled Q chunk (DoubleRow moving)."""
                    csl = slice(qc * QC, (qc + 1) * QC)
                    for co in range(2):
                        nc.vector.tensor_copy(qnt8[:, co, csl],
                                              qnt[co][:, csl])

                qproj_a(0)
                qproj_b(0)
                qproj_c(0)
                qproj_d(0)

                def denom(qc):
                    """Softmax denominator for chunk qc (emitted a few
                    iterations into chunk qc+1)."""
                    racc, msb, esb = state[qc]
                    ps_r = psl.tile([1, QC], F32, name="ps_r", tag="small")
                    nc.tensor.matmul(ps_r[:], ones_c16[:], racc[:])
                    lnr = w2.tile([1, QC], F32, name="lnr", bufs=1)
                    nc.scalar.activation(lnr[:], ps_r[:], ACTF.Ln)
                    rinv_row = w2.tile([1, QC], F16, name="rinv_row",
                                       bufs=1)
                    nc.scalar.activation(rinv_row[:], lnr[:], ACTF.Exp,
                                         scale=-1.0)
                    ps_rb = psa.tile([128, QC], F32, name="ps_rb")
                    nc.tensor.matmul(ps_rb[:], ones_r16[:], rinv_row[:])
                    rinv = w2.tile([128, QC], F16, name="rinv", bufs=2)
                    nc.vector.tensor_copy(rinv[:], ps_rb[:])
                    state[qc] = (racc, msb, esb, rinv)

                def epilogue_ci(qc, ci):
                    _, msb, esb, rinv = state[qc]
                    qsl = slice(qc * QC, (qc + 1) * QC)
                    mhat = w2.tile([128, QC], F16, name="mhat", bufs=2)
                    nc.vector.tensor_mul(mhat[:], msb[ci][:], rinv[:])
                    ehat = w2.tile([128, QC], F16, name="ehat", bufs=2)
                    nc.vector.tensor_mul(ehat[:], esb[ci][:], rinv[:])
                    s2p = w2.tile([128, QC], F16, name="s2p", bufs=2)
                    nc.scalar.activation(s2p[:], mhat[:], ACTF.Square)
                    s2 = w2.tile([128, QC], F16, name="s2", bufs=2)
                    nc.vector.tensor_sub(s2[:], ehat[:], s2p[:])
                    nc.vector.tensor_scalar_max(s2[:], s2[:], 0.0)
                    # sqrt(s2) = Exp(0.5*Ln(s2+tiny)); ln stays fp32 (its
                    # absolute error is amplified by the exp)
                    ln2 = w2.tile([128, QC], F32, name="ln2", bufs=2)
                    nc.scalar.activation(ln2[:], s2[:], ACTF.Ln,
                                         bias=eps_ln_t[:])
                    s_sb = w2.tile([128, QC], F16, name="s_sb", bufs=2)
                    nc.scalar.activation(s_sb[:], ln2[:], ACTF.Exp,
                                         scale=0.5)
                    o_sb = w2.tile([128, QC], F16, name="o_sb", bufs=2)
                    nc.vector.tensor_mul(o_sb[:], s_sb[:], nct[ci][:, qsl])
                    o_f = w2.tile([128, QC], F32, name="o_f", bufs=2)
                    nc.vector.tensor_add(o_f[:], o_sb[:], mhat[:])
                    nc.sync.dma_start(
                        out_e[ci * 128:(ci + 1) * 128, qsl], o_f[:]
                    )
                    if ci == 1:
                        state.pop(qc)

                for qc in range(NQC):
                    qsl = slice(qc * QC, (qc + 1) * QC)
                    ps_m = [psa.tile([128, QC], F32, name=f"ps_m{c}")
                            for c in range(2)]
                    ps_e = [psa.tile([128, QC], F32, name=f"ps_e{c}")
                            for c in range(2)]
                    racc = w2.tile([128, QC], F16, name="racc")

                    def emit_av(pr, p8):
                        first, last = pr == 0, pr == NPR - 1
                        for ci in range(2):
                            cs = slice(ci * 128, (ci + 1) * 128)
                            nc.tensor.matmul(ps_m[ci][:],
                                             v8[:, pr, :, cs], p8[:],
                                             start=first, stop=last,
                                             perf_mode=PM.DoubleRow)
                            nc.tensor.matmul(ps_e[ci][:],
                                             v28[:, pr, :, cs], p8[:],
                                             start=first, stop=last,
                                             perf_mode=PM.DoubleRow)

                    pend = None
                    for pr in range(NPR):
                        pair = ppx.tile([128, 2 * QC], F32, name="pair")
                        for wh in range(2):
                            kt = 2 * pr + wh
                            ksl = slice(kt * 128, (kt + 1) * 128)
                            nc.tensor.matmul(pair[:, wh * QC:(wh + 1) * QC],
                                             knt8[:, :, ksl],
                                             qnt8[:, :, qsl],
                                             start=True, stop=True,
                                             perf_mode=PM.DoubleRow)
                        p8 = w2.tile([128, 2, QC], FP8, name="p8", bufs=4)
                        nc.scalar.activation(p8[:], pair[:], ACTF.Exp)
                        padd = w2.tile([128, QC], F16, name="padd", bufs=3)
                        nc.gpsimd.tensor_add(padd[:], p8[:, 0, :],
                                             p8[:, 1, :])
                        if pr == 0:
                            nc.vector.tensor_copy(racc[:], padd[:])
                        else:
                            nc.vector.tensor_add(racc[:], racc[:], padd[:])
                        if qc > 0:
                            if pr == 1:
                                denom(qc - 1)
                            elif pr == 3:
                                epilogue_ci(qc - 1, 0)
                            elif pr == 5:
                                epilogue_ci(qc - 1, 1)
                        if qc + 1 < NQC:
                            if pr == 8:
                                qproj_a(qc + 1)
                            elif pr == 10:
                                qproj_b(qc + 1)
                            elif pr == 12:
                                qproj_c(qc + 1)
                            elif pr == 13:
                                qproj_d(qc + 1)
                        if pend is not None:
                            emit_av(*pend)
                        pend = (pr, p8)
                        if qc == NQC - 1 and pr == NPR - 1:
                            state[qc] = (racc, None, None)
                            denom(qc)
                            dstate = state.pop(qc)
                    emit_av(*pend)
                    # evacuate accumulators fast (ACT) to free PSUM banks
                    msb = [w2.tile([128, QC], F16, name=f"msb{c}")
                           for c in range(2)]
                    esb = [w2.tile([128, QC], F16, name=f"esb{c}")
                           for c in range(2)]
                    for ci in range(2):
                        nc.scalar.activation(msb[ci][:], ps_m[ci][:],
                                             ACTF.Copy)
                        nc.scalar.activation(esb[ci][:], ps_e[ci][:],
                                             ACTF.Copy)
                    if qc == NQC - 1:
                        state[qc] = (dstate[0], msb, esb, dstate[3])
                    else:
                        state[qc] = (racc, msb, esb)
                epilogue_ci(NQC - 1, 0)
                epilogue_ci(NQC - 1, 1)

    _legalize_waits(nc)
    return nc


_NC_CACHE = {}


def _get_nc():
    if "nc" not in _NC_CACHE:
        _NC_CACHE["nc"] = build_nc()
    return _NC_CACHE["nc"]


def kernel(content, style, Wq, bq, Wk, bk, Wv, bv):
    content = np.asarray(content, dtype=np.float32)
    style = np.asarray(style, dtype=np.float32)
    Wq8 = _pack_pairs(np.asarray(Wq, dtype=np.float32))
    Wkb = np.ascontiguousarray(np.asarray(Wk, dtype=np.float32)).astype(NPBF16)
    Wv8 = _pack_pairs(np.asarray(Wv, dtype=np.float32))
    bqr = np.asarray(bq, dtype=np.float32).reshape(1, C)
    bkr = np.asarray(bk, dtype=np.float32).reshape(1, C)
    bvr = np.asarray(bv, dtype=np.float32).reshape(1, C).astype(NPBF16)

    nc = _get_nc()
    in_maps = []
    for core in range(8):
        b, h = core // 2, core % 2
        xt = np.ascontiguousarray(content[b].reshape(N, C).T).astype(NPBF16)
        st8 = _pack_pairs(style[b].reshape(N, C).T)
        xa = np.ascontiguousarray(xt[:, h * QH:(h + 1) * QH])
        xb = np.ascontiguousarray(xt[:, (1 - h) * QH:(2 - h) * QH])
        in_maps.append({
            "xa": xa, "xb": xb, "st": st8,
            "wq": Wq8, "wk": Wkb, "wv": Wv8,
            "bqr": bqr, "bkr": bkr, "bvr": bvr,
        })

    trace = os.environ.get("BASS_KERNEL_TRACE", "0") == "1"
    if trace:
        _install_profshim()
    res = run_bass_kernel_spmd(nc, in_maps, list(range(8)), trace=trace)
    LAST_EXEC_NS["v"] = res.exec_time_ns

    out = np.empty((B, H, W, C), dtype=np.float32)
    for core in range(8):
        b, h = core // 2, core % 2
        o = res.results[core]["out"]          # [C, QH]
        out[b].reshape(N, C)[h * QH:(h + 1) * QH, :] = o.T
    return out


# revision 15
# speedup vs baseline: 1.0228x; 1.0228x over previous
"""AdaptiveAttentionLayer on 8 TRN2 NeuronCores.

Full inputs in, full output out. Sharding: data-parallel over batch (B=4)
x 2-way sequence-parallel over the 4096 query rows -> 8 cores, each core
computes a [2048, 256] slice of one batch item's output.

The PE streams moving data at ~1 row/cycle regardless of dtype, so the
only matmul lever is fewer rows: the attention core (scores, A@V,
A@V^2 - 87% of PE work) runs as fp8e4 DoubleRow matmuls, which pack two
128-deep contractions per pass (2x). K^T is pre-normalized (1/||k||
folded in) so the softmax exp needs no per-key scale and one fused Exp
covers a 2-bank PSUM score pair. Softmax denominators: GpSimd sums each
fp8 P pair into fp16, DVE accumulates fp16 at its 4x perf mode. All
sqrt/rsqrt/reciprocal are Ln+Exp compositions so the scalar engine
keeps ONE activation table loaded (ln/exp/square/copy). PSUM plan:
score-pair 2 banks + M/E2 accumulators 4 + broadcast 1 + small rows 1.

Per-core device pipeline (channel-major / transposed layouts):
  - instance-norm stats of content/style (free-axis reductions)
  - V = style @ Wv   row-major; bias broadcast-added; V2=V*V (fp8 pairs)
  - K^T = (diag(inv_s) Wk)^T style^T + bias  channel-major bf16,
    column-l2-normalized via PE colsums + Ln/Exp + PE broadcast -> fp8
  - Q^T = Wq^T norm_content^T, l2norm likewise -> fp8
  - scores^T pair = Khat_pair qhat (fp8 DoubleRow, 512-query chunks)
  - P = exp(scores) fused per pair -> fp8
  - M^T = V^T P^T, E2^T = (V*V)^T P^T (fp8 DoubleRow, PSUM-accumulated)
  - r = sum_k P (GpSimd pair adds + DVE fp16 + PE ones-matmul),
    out = sqrt(relu(E2/r-(M/r)^2)) * norm_content + M/r
"""

import sys

if "/opt/trn_rl_repo" not in sys.path:
    sys.path.insert(0, "/opt/trn_rl_repo")

import os
import numpy as np
import ml_dtypes

import concourse.bass as bass
import concourse.mybir as mybir
import concourse.tile as tile
from concourse.bass_utils import run_bass_kernel_spmd

F32 = mybir.dt.float32
BF16 = mybir.dt.bfloat16
F16 = mybir.dt.float16
FP8 = mybir.dt.float8e4
PM = mybir.MatmulPerfMode
ALU = mybir.AluOpType
ACTF = mybir.ActivationFunctionType

B, H, W, C = 4, 64, 64, 256
N = H * W          # 4096 key/query rows per batch item
QH = N // 2        # 2048 query rows per core
NK = N // 128      # 32 key tiles
NPR = NK // 2      # 16 key-tile pairs (fp8 DoubleRow)
QC = 512           # query chunk (matmul moving free dim)
NQC = QH // QC     # 4 query chunks per core
EPS_IN = 1e-5      # instance norm eps
EPS_L2 = 1e-12     # l2norm eps
EPS_LN = 1e-30     # guards Ln(0) in sqrt-by-Ln/Exp

LAST_EXEC_NS = {"v": None}

NPBF16 = ml_dtypes.bfloat16
NPFP8 = mybir.dt.np(FP8)


def _pack_pairs(a):
    """[256, F] -> [128, 2*F] fp8 pair layout (dim1 = which 128-half)."""
    f = a.shape[1]
    return np.ascontiguousarray(
        a.reshape(2, 128, f).transpose(1, 0, 2).reshape(128, 2 * f)
    ).astype(NPFP8)


def _legalize_waits(nc):
    """This walrus build accepts at most ONE sync wait per instruction
    ('Too many sync wait commands'). Hoist extra waits onto same-engine
    NOPs inserted immediately before the offending instruction."""
    fn = nc.m.functions[0]
    nfix = 0
    for bb in fn.blocks:
        i = 0
        while i < len(bb.instructions):
            inst = bb.instructions[i]
            si = inst.sync_info
            if si is not None and len(si.on_wait) > 1:
                waits = list(si.on_wait)
                for j, w in enumerate(waits[:-1]):
                    nop = mybir.InstNoOp(
                        name=nc.get_next_instruction_name(), ins=[], outs=[]
                    )
                    nop.engine = inst.engine
                    nop.sync_info = mybir.SyncInfo(on_wait=[w], on_update=[])
                    nc.register_instruction(nop)
                    bb.instructions.insert(i + j, nop)
                i += len(waits) - 1
                inst.sync_info = mybir.SyncInfo(
                    on_wait=[waits[-1]], on_update=list(si.on_update)
                )
                nfix += 1
            i += 1
    return nfix


def _install_profshim():
    """antenv.axon_hooks is absent in this image; provide it (ctypes into
    libaxon_pjrt.so) plus an offline-safe upload_artifacts so trace=True
    yields exec_time_ns."""
    import contextlib, ctypes, types

    if "antenv.axon_hooks" in sys.modules:
        return
    so = "/opt/axon/libaxon_pjrt.so"
    hook = None
    if os.path.exists(so):
        lib = ctypes.CDLL(so)
        if hasattr(lib, "axon_start_nrt_profile"):
            lib.axon_start_nrt_profile.argtypes = [
                ctypes.POINTER(ctypes.c_int64),
                ctypes.c_size_t,
            ]
            lib.axon_start_nrt_profile.restype = ctypes.c_int64
            lib.axon_stop_nrt_profile.argtypes = [ctypes.c_char_p]
            lib.axon_stop_nrt_profile.restype = ctypes.c_int64

            @contextlib.contextmanager
            def _hook(output_dir, device_ids):
                import jax

                jax.devices()
                if device_ids:
                    ids = (ctypes.c_int64 * len(device_ids))(*device_ids)
                    rc = lib.axon_start_nrt_profile(ids, len(device_ids))
                else:
                    rc = lib.axon_start_nrt_profile(None, 0)
                if rc != 0:
                    raise RuntimeError(f"axon_start_nrt_profile rc={rc}")
                try:
                    yield
                finally:
                    n = lib.axon_stop_nrt_profile(str(output_dir).encode())
                    print(f"profile: {n} ntff file(s) -> {output_dir}",
                          file=sys.stderr)

            hook = _hook

    mod = types.ModuleType("antenv.axon_hooks")
    mod.get_axon_ntff_profile_hook = lambda: hook
    mod.set_axon_ntff_profile_hook = lambda h: None
    sys.modules["antenv.axon_hooks"] = mod

    import concourse.bass_utils as bu

    bu.upload_artifacts = lambda tmpdir: tmpdir


def build_nc():
    nc = bass.Bass()

    xa_e = nc.declare_dram_parameter("xa", [C, QH], BF16, isOutput=False)
    xb_e = nc.declare_dram_parameter("xb", [C, QH], BF16, isOutput=False)
    st_e = nc.declare_dram_parameter("st", [128, 2 * N], FP8, isOutput=False)
    wq_e = nc.declare_dram_parameter("wq", [128, 2 * C], FP8, isOutput=False)
    wk_e = nc.declare_dram_parameter("wk", [C, C], BF16, isOutput=False)
    wv_e = nc.declare_dram_parameter("wv", [128, 2 * C], FP8, isOutput=False)
    bqr_e = nc.declare_dram_parameter("bqr", [C, 1], F32, isOutput=False)
    bkr_e = nc.declare_dram_parameter("bkr", [C, 1], F32, isOutput=False)
    bvr_e = nc.declare_dram_parameter("bvr", [1, C], BF16, isOutput=False)
    out_e = nc.declare_dram_parameter("out", [C, QH], F32, isOutput=True)

    NCH_K = N // QC       # 8 key chunks
    DCH = 1024
    SCH = 2048            # stats chunk

    with tile.TileContext(nc) as tc, \
            nc.allow_low_precision(reason="fp8/bf16 attention pipeline"):
        with tc.tile_pool(name="persist", bufs=1) as pp:
            ones_col = pp.tile([128, 1], BF16)  # colsum stationary
            ones_c16 = pp.tile([128, 1], F16)   # denom colsum stationary
            ones_rbf = pp.tile([1, 128], BF16)  # bv broadcast stationary
            ones_r16 = pp.tile([1, 128], F16)   # rinv/iqr/invk broadcast
            eps_in_t = pp.tile([128, 1], F32)
            eps_l2_t = pp.tile([128, 1], F32)
            eps_ln_t = pp.tile([128, 1], F32)
            wq8 = pp.tile([128, 2, C], FP8)
            wk_s = [pp.tile([128, C], BF16, name=f"wk{i}") for i in range(2)]
            wk8 = pp.tile([128, 2, C], FP8)
            wv8 = pp.tile([128, 2, C], FP8)
            nct8 = pp.tile([128, 2, QH], FP8)
            bqc = [pp.tile([128, 1], F32, name=f"bqc{i}") for i in range(2)]
            bkc = [pp.tile([128, 1], F32, name=f"bkc{i}") for i in range(2)]
            bkc_f = [pp.tile([128, 1], F32, name=f"bkf{i}") for i in range(2)]
            bv_row = pp.tile([1, C], BF16)
            bvb = pp.tile([128, C], F32)
            # DoubleRow pair layouts (dim1 = which half of the 256-deep
            # contraction):
            #   knt8[:, co, k]      Khat^T chans co*128.., key k
            #   qnt8[:, co, q]      Qhat^T chans co*128..
            #   v8[:, pr, w, c]     V[key tile 2pr+w, chan c]
            knt_bf = pp.tile([128, 2, N], BF16)
            knt8 = pp.tile([128, 2, N], FP8)
            qnt8 = pp.tile([128, 2, QH], FP8)
            qnt = [pp.tile([128, QH], BF16, name=f"qnt{i}") for i in range(2)]
            nct = [pp.tile([128, QH], BF16, name=f"nct{i}") for i in range(2)]
            v8 = pp.tile([128, NPR, 2, C], FP8)
            v28 = pp.tile([128, NPR, 2, C], FP8)
            mean_s = [pp.tile([128, 1], F32, name=f"ms{i}") for i in range(2)]
            inv_s = [pp.tile([128, 1], F32, name=f"is{i}") for i in range(2)]
            mean_x = [pp.tile([128, 1], F32, name=f"mx{i}") for i in range(2)]
            inv_x = [pp.tile([128, 1], F32, name=f"ix{i}") for i in range(2)]

            nc.vector.memset(ones_col[:], 1.0)
            nc.vector.memset(ones_c16[:], 1.0)
            nc.vector.memset(ones_rbf[:], 1.0)
            nc.vector.memset(ones_r16[:], 1.0)
            nc.vector.memset(eps_in_t[:], EPS_IN)
            nc.vector.memset(eps_l2_t[:], EPS_L2)
            nc.vector.memset(eps_ln_t[:], EPS_LN)

            # ================= phase 1: stats + projections =================
            with (
                tc.tile_pool(name="inputs", bufs=1) as tp,
                tc.tile_pool(name="w1", bufs=2) as w1,
                tc.tile_pool(name="psum1", bufs=3, space="PSUM") as ps1,
            ):
                st8 = tp.tile([128, 2, N], FP8, name="st8")
                xa_t = [tp.tile([128, QH], BF16, name=f"xa{i}")
                        for i in range(2)]
                for w in range(2):
                    nc.sync.dma_start(wv8[:, w, :], wv_e[:, w * C:(w + 1) * C])
                    nc.sync.dma_start(wq8[:, w, :], wq_e[:, w * C:(w + 1) * C])
                for i in range(2):
                    nc.sync.dma_start(wk_s[i][:],
                                      wk_e[i * 128:(i + 1) * 128, :])
                    nc.sync.dma_start(bqc[i][:], bqr_e[i * 128:(i + 1) * 128, :])
                    nc.sync.dma_start(bkc[i][:], bkr_e[i * 128:(i + 1) * 128, :])
                nc.sync.dma_start(bv_row[:], bvr_e[:])
                for j in range(0, N, DCH):
                    for i in range(2):
                        nc.sync.dma_start(
                            st8[:, i, j:j + DCH],
                            st_e[:, i * N + j:i * N + j + DCH])
                for j in range(0, QH, DCH):
                    for i in range(2):
                        nc.sync.dma_start(
                            xa_t[i][:, j:j + DCH],
                            xa_e[i * 128:(i + 1) * 128, j:j + DCH],
                        )

                # bv broadcast for V row-major bias add
                ps_bc = ps1.tile([128, C], F32, name="ps_bc", tag="prj")
                nc.tensor.matmul(ps_bc[:], ones_rbf[:], bv_row[:])
                nc.vector.tensor_copy(bvb[:], ps_bc[:])

                def stats_closures(chunks, mean, inv, i):
                    """Return a list of closures; call them in order, spaced
                    between PE-heavy work. Last closure finalizes stats."""
                    nck = len(chunks)
                    parts = w1.tile([128, nck], F32, name="parts",
                                    bufs=2)
                    parts_m = w1.tile([128, nck], F16, name="parts_m",
                                      bufs=2)
                    out = []

                    def chunk_op(j, ch):
                        def go():
                            scr = w1.tile([128, N], BF16, name="sqscr",
                                          bufs=2)
                            nc.scalar.activation(
                                scr[:, 0:ch.free_size()], ch, ACTF.Square,
                                accum_out=parts[:, j:j + 1],
                            )
                            nc.vector.tensor_reduce(
                                parts_m[:, j:j + 1], ch,
                                axis=mybir.AxisListType.X, op=ALU.add,
                            )
                        return go

                    for j, ch in enumerate(chunks):
                        out.append(chunk_op(j, ch))

                    def finalize():
                        ssq = w1.tile([128, 1], F32, name="ssq")
                        nc.vector.reduce_sum(ssq[:], parts[:, 0:nck],
                                             axis=mybir.AxisListType.X)
                        ssum = w1.tile([128, 1], F32, name="ssum")
                        nc.vector.reduce_sum(ssum[:], parts_m[:, 0:nck],
                                             axis=mybir.AxisListType.X)
                        nc.vector.tensor_scalar_mul(mean[i][:], ssum[:],
                                                    1.0 / N)
                        ex2 = w1.tile([128, 1], F32, name="ex2")
                        nc.vector.tensor_scalar_mul(ex2[:], ssq[:], 1.0 / N)
                        msq = w1.tile([128, 1], F32, name="msq")
                        nc.vector.tensor_mul(msq[:], mean[i][:], mean[i][:])
                        var = w1.tile([128, 1], F32, name="var")
                        nc.vector.tensor_sub(var[:], ex2[:], msq[:])
                        # 1/sqrt(var+eps) = Exp(-0.5*Ln(var+eps))
                        lnv = w1.tile([128, 1], F32, name="lnv")
                        nc.scalar.activation(lnv[:], var[:], ACTF.Ln,
                                             bias=eps_in_t[:])
                        nc.scalar.activation(inv[i][:], lnv[:], ACTF.Exp,
                                             scale=-0.5)
                    out.append(finalize)
                    return out

                style_ops = []
                for i in range(2):
                    style_ops += stats_closures([st8[:, i, :]],
                                                mean_s, inv_s, i)

                # ---- V projection (row-major; bias added at evacuation
                # straight into the fp8 pair layout); V2 = V*V behind it.
                # style-stats ops interleaved so they don't head-of-line
                # block the V PSUM evacuations
                for kt in range(NK):
                    ksl = slice(kt * 128, (kt + 1) * 128)
                    ps_v = ps1.tile([128, C], F32, name="ps_v", tag="prj")
                    nc.tensor.matmul(ps_v[:], st8[:, :, ksl], wv8[:],
                                     start=True, stop=True,
                                     perf_mode=PM.DoubleRow)
                    vdst = v8[:, kt // 2, kt % 2, :]
                    nc.vector.tensor_add(vdst, ps_v[:], bvb[:])
                    nc.gpsimd.tensor_mul(v28[:, kt // 2, kt % 2, :],
                                         vdst, vdst)
                    if kt % 3 == 2 and style_ops:
                        style_ops.pop(0)()
                while style_ops:
                    style_ops.pop(0)()

                # ---- fold style instance norm into Wk; column bias corr
                for i in range(2):
                    nc.vector.tensor_scalar_mul(wk_s[i][:], wk_s[i][:],
                                                inv_s[i][:])
                mu_inv = [w1.tile([128, 1], BF16, name=f"mi{i}")
                          for i in range(2)]
                for i in range(2):
                    nc.vector.tensor_mul(mu_inv[i][:], mean_s[i][:],
                                         inv_s[i][:])
                for co in range(2):
                    ps_c = ps1.tile([128, 1], F32, name="ps_c", tag="pn",
                                    bufs=2)
                    csl = slice(co * 128, (co + 1) * 128)
                    nc.tensor.matmul(ps_c[:], wk_s[0][:, csl],
                                     mu_inv[0][:], start=True, stop=False)
                    nc.tensor.matmul(ps_c[:], wk_s[1][:, csl],
                                     mu_inv[1][:], start=False, stop=True)
                    nc.vector.tensor_sub(bkc_f[co][:], bkc[co][:], ps_c[:])
                for w in range(2):
                    nc.vector.tensor_copy(wk8[:, w, :], wk_s[w][:])

                # ---- K^T projection (channel-major bf16) + column
                # sumsq + per-chunk l2 normalization into fp8 (pipelined
                # so the norm chain hides under later chunks' matmuls)

                def proj_t(src, w_t, bias_c, nch, interleave=None):
                    def colsum(ch, sq):
                        csl = slice(ch * QC, (ch + 1) * QC)
                        ps_n = ps1.tile([1, QC], F32, name="ps_n", tag="pn",
                                        bufs=2)
                        nc.tensor.matmul(ps_n[:], ones_col[:],
                                         sq[0][:], start=True, stop=False)
                        nc.tensor.matmul(ps_n[:], ones_col[:],
                                         sq[1][:], start=False, stop=True)
                        lnk = w1.tile([1, QC], F32, name="lnk", bufs=2)
                        nc.scalar.activation(lnk[:], ps_n[:], ACTF.Ln,
                                             bias=eps_l2_t[0:1, :])
                        ivr = w1.tile([1, QC], F16, name="ivr", bufs=2)
                        nc.scalar.activation(ivr[:], lnk[:], ACTF.Exp,
                                             scale=-0.5)
                        ps_b = ps1.tile([128, QC], F32, name="ps_b",
                                        tag="pbig")
                        nc.tensor.matmul(ps_b[:], ones_r16[:], ivr[:])
                        for co in range(2):
                            nc.vector.tensor_mul(knt8[:, co, csl],
                                                 knt_bf[:, co, csl], ps_b[:])

                    pend = None
                    for ch in range(nch):
                        csl = slice(ch * QC, (ch + 1) * QC)
                        sq = []
                        for co in range(2):
                            wsl = slice(co * 128, (co + 1) * 128)
                            ps_p = ps1.tile([128, QC], F32, name="ps_p",
                                            tag="pbig")
                            nc.tensor.matmul(ps_p[:], w_t[:, :, wsl],
                                             src[:, :, csl],
                                             start=True, stop=True,
                                             perf_mode=PM.DoubleRow)
                            kdst = knt_bf[:, co, csl]
                            nc.vector.tensor_scalar(
                                out=kdst, in0=ps_p[:],
                                scalar1=bias_c[co][:], scalar2=None,
                                op0=ALU.add)
                            s = w1.tile([128, QC], BF16, name="sqc", bufs=3)
                            nc.scalar.activation(s[:], ps_p[:], ACTF.Square,
                                                 bias=bias_c[co][:])
                            sq.append(s)
                        if pend is not None:
                            colsum(*pend)
                        pend = (ch, sq)
                        if interleave:
                            interleave.pop(0)()
                    colsum(*pend)
                    while interleave:
                        interleave.pop(0)()

                # content stats + norm_content interleaved into the K
                # projection (they wait on the later xa/xb DMA; queuing
                # them first would stall the K chain's engines)
                xbch = {}
                for i in range(2):
                    cb = tp.tile([128, QH], BF16, name="xbs", bufs=2)
                    nc.sync.dma_start(cb[:], xb_e[i * 128:(i + 1) * 128, :])
                    xbch[i] = cb
                content_ops = []
                for i in range(2):
                    content_ops += stats_closures([xa_t[i][:], xbch[i][:]],
                                                  mean_x, inv_x, i)

                def nct_op(i):
                    def go():
                        nc.vector.tensor_scalar(
                            out=nct[i][:], in0=xa_t[i][:],
                            scalar1=mean_x[i][:], scalar2=inv_x[i][:],
                            op0=ALU.subtract, op1=ALU.mult,
                        )
                        nc.vector.tensor_copy(nct8[:, i, :], nct[i][:])
                    return go

                content_ops += [nct_op(0), nct_op(1)]
                proj_t(st8, wk8, bkc_f, NCH_K, content_ops)


            # ========== phase 2: attention ==========
            with (
                tc.tile_pool(name="w2", bufs=2) as w2,
                tc.tile_pool(name="psum_acc", bufs=1, space="PSUM") as psa,
                tc.tile_pool(name="psum_pair", bufs=1, space="PSUM") as ppx,
                tc.tile_pool(name="psum_small", bufs=1, space="PSUM") as psl,
            ):
                state = {}
                qstate = {}

                def qproj_a(qc):
                    """Project Q chunk qc into qnt (channel-major) and
                    square for column norms."""
                    csl = slice(qc * QC, (qc + 1) * QC)
                    pq = ppx.tile([128, 2 * QC], F32, name="pair")
                    for co in range(2):
                        wsl = slice(co * 128, (co + 1) * 128)
                        osl = slice(co * QC, (co + 1) * QC)
                        nc.tensor.matmul(pq[:, osl], wq8[:, :, wsl],
                                         nct8[:, :, csl],
                                         start=True, stop=True,
                                         perf_mode=PM.DoubleRow)
                    sq = []
                    for co in range(2):
                        osl = slice(co * QC, (co + 1) * QC)
                        nc.vector.tensor_scalar(
                            out=qnt[co][:, csl], in0=pq[:, osl],
                            scalar1=bqc[co][:], scalar2=None, op0=ALU.add)
                        s = w2.tile([128, QC], BF16, name="qsq", bufs=2)
                        nc.scalar.activation(s[:], qnt[co][:, csl],
                                             ACTF.Square)
                        sq.append(s)
                    qstate[qc] = sq

                def qproj_b(qc):
                    """Column sumsq -> 1/norm row for chunk qc."""
                    sq = qstate.pop(qc)
                    ps_n = psl.tile([1, QC], F32, name="ps_n", tag="small")
                    nc.tensor.matmul(ps_n[:], ones_col[:], sq[0][:],
                                     start=True, stop=False)
                    nc.tensor.matmul(ps_n[:], ones_col[:], sq[1][:],
                                     start=False, stop=True)
                    lnq = w2.tile([1, QC], F32, name="lnq", bufs=1)
                    nc.scalar.activation(lnq[:], ps_n[:], ACTF.Ln,
                                         bias=eps_l2_t[0:1, :])
                    iqr = w2.tile([1, QC], F16, name="invr", bufs=2)
                    nc.scalar.activation(iqr[:], lnq[:], ACTF.Exp,
                                         scale=-0.5)
                    qstate[qc] = iqr

                def qproj_c(qc):
                    """Broadcast 1/norm and scale Q chunk qc columns."""
                    iqr = qstate.pop(qc)
                    csl = slice(qc * QC, (qc + 1) * QC)
                    ps_b = psa.tile([128, QC], F32, name="qps_b",
                                    tag="ps_rb")
                    nc.tensor.matmul(ps_b[:], ones_r16[:], iqr[:])
                    for co in range(2):
                        nc.vector.tensor_mul(qnt[co][:, csl],
                                             qnt[co][:, csl], ps_b[:])

                def qproj_d(qc):
                    """fp8 copy of the scaled Q chunk (DoubleRow moving)."""
                    csl = slice(qc * QC, (qc + 1) * QC)
                    for co in range(2):
                        nc.vector.tensor_copy(qnt8[:, co, csl],
                                              qnt[co][:, csl])

                qproj_a(0)
                qproj_b(0)
                qproj_c(0)
                qproj_d(0)

                def denom(qc):
                    """Softmax denominator for chunk qc (emitted a few
                    iterations into chunk qc+1)."""
                    racc, msb, esb = state[qc]
                    ps_r = psl.tile([1, QC], F32, name="ps_r", tag="small")
                    nc.tensor.matmul(ps_r[:], ones_c16[:], racc[:])
                    lnr = w2.tile([1, QC], F32, name="lnr", bufs=1)
                    nc.scalar.activation(lnr[:], ps_r[:], ACTF.Ln)
                    rinv_row = w2.tile([1, QC], F16, name="rinv_row",
                                       bufs=1)
                    nc.scalar.activation(rinv_row[:], lnr[:], ACTF.Exp,
                                         scale=-1.0)
                    ps_rb = psa.tile([128, QC], F32, name="ps_rb")
                    nc.tensor.matmul(ps_rb[:], ones_r16[:], rinv_row[:])
                    rinv = w2.tile([128, QC], F16, name="rinv", bufs=2)
                    nc.vector.tensor_copy(rinv[:], ps_rb[:])
                    state[qc] = (racc, msb, esb, rinv)

                def epilogue_ci(qc, ci):
                    _, msb, esb, rinv = state[qc]
                    qsl = slice(qc * QC, (qc + 1) * QC)
                    mhat = w2.tile([128, QC], F16, name="mhat", bufs=2)
                    nc.vector.tensor_mul(mhat[:], msb[ci][:], rinv[:])
                    ehat = w2.tile([128, QC], F16, name="ehat", bufs=2)
                    nc.vector.tensor_mul(ehat[:], esb[ci][:], rinv[:])
                    s2p = w2.tile([128, QC], F16, name="s2p", bufs=2)
                    nc.scalar.activation(s2p[:], mhat[:], ACTF.Square)
                    s2 = w2.tile([128, QC], F16, name="s2", bufs=2)
                    nc.vector.tensor_sub(s2[:], ehat[:], s2p[:])
                    nc.vector.tensor_scalar_max(s2[:], s2[:], 0.0)
                    # sqrt(s2) = Exp(0.5*Ln(s2+tiny)); ln stays fp32 (its
                    # absolute error is amplified by the exp)
                    ln2 = w2.tile([128, QC], F32, name="ln2", bufs=2)
                    nc.scalar.activation(ln2[:], s2[:], ACTF.Ln,
                                         bias=eps_ln_t[:])
                    s_sb = w2.tile([128, QC], F16, name="s_sb", bufs=2)
                    nc.scalar.activation(s_sb[:], ln2[:], ACTF.Exp,
                                         scale=0.5)
                    o_sb = w2.tile([128, QC], F16, name="o_sb", bufs=2)
                    nc.vector.tensor_mul(o_sb[:], s_sb[:], nct[ci][:, qsl])
                    o_f = w2.tile([128, QC], F32, name="o_f", bufs=2)
                    nc.vector.tensor_add(o_f[:], o_sb[:], mhat[:])
                    nc.sync.dma_start(
                        out_e[ci * 128:(ci + 1) * 128, qsl], o_f[:]
                    )
                    if ci == 1:
                        state.pop(qc)

                for qc in range(NQC):
                    qsl = slice(qc * QC, (qc + 1) * QC)
                    ps_m = [psa.tile([128, QC], F32, name=f"ps_m{c}")
                            for c in range(2)]
                    ps_e = [psa.tile([128, QC], F32, name=f"ps_e{c}")
                            for c in range(2)]
                    racc = w2.tile([128, QC], F16, name="racc")

                    def emit_av(pr, p8):
                        first, last = pr == 0, pr == NPR - 1
                        for ci in range(2):
                            cs = slice(ci * 128, (ci + 1) * 128)
                            nc.tensor.matmul(ps_m[ci][:],
                                             v8[:, pr, :, cs], p8[:],
                                             start=first, stop=last,
                                             perf_mode=PM.DoubleRow)
                            nc.tensor.matmul(ps_e[ci][:],
                                             v28[:, pr, :, cs], p8[:],
                                             start=first, stop=last,
                                             perf_mode=PM.DoubleRow)

                    pend = None
                    for pr in range(NPR):
                        pair = ppx.tile([128, 2 * QC], F32, name="pair")
                        for wh in range(2):
                            kt = 2 * pr + wh
                            ksl = slice(kt * 128, (kt + 1) * 128)
                            nc.tensor.matmul(pair[:, wh * QC:(wh + 1) * QC],
                                             knt8[:, :, ksl],
                                             qnt8[:, :, qsl],
                                             start=True, stop=True,
                                             perf_mode=PM.DoubleRow)
                        p8 = w2.tile([128, 2, QC], FP8, name="p8", bufs=4)
                        nc.scalar.activation(p8[:], pair[:], ACTF.Exp)
                        padd = w2.tile([128, QC], F16, name="padd", bufs=3)
                        nc.gpsimd.tensor_add(padd[:], p8[:, 0, :],
                                             p8[:, 1, :])
                        if pr == 0:
                            nc.vector.tensor_copy(racc[:], padd[:])
                        else:
                            nc.vector.tensor_add(racc[:], racc[:], padd[:])
                        if qc > 0:
                            if pr == 1:
                                denom(qc - 1)
                            elif pr == 3:
                                epilogue_ci(qc - 1, 0)
                            elif pr == 5:
                                epilogue_ci(qc - 1, 1)
                        if qc + 1 < NQC:
                            if pr == 8:
                                qproj_a(qc + 1)
                            elif pr == 10:
                                qproj_b(qc + 1)
                            elif pr == 12:
                                qproj_c(qc + 1)
                            elif pr == 13:
                                qproj_d(qc + 1)
                        if pend is not None:
                            emit_av(*pend)
                        pend = (pr, p8)
                        if qc == NQC - 1 and pr == NPR - 1:
                            state[qc] = (racc, None, None)
                            denom(qc)
                            dstate = state.pop(qc)
                    emit_av(*pend)
                    # evacuate accumulators fast (ACT) to free PSUM banks
                    msb = [w2.tile([128, QC], F16, name=f"msb{c}")
                           for c in range(2)]
                    esb = [w2.tile([128, QC], F16, name=f"esb{c}")
                           for c in range(2)]
                    for ci in range(2):
                        nc.scalar.activation(msb[ci][:], ps_m[ci][:],
                                             ACTF.Copy)
                        nc.scalar.activation(esb[ci][:], ps_e[ci][:],
                                             ACTF.Copy)
                    if qc == NQC - 1:
                        state[qc] = (dstate[0], msb, esb, dstate[3])
                    else:
                        state[qc] = (racc, msb, esb)
                epilogue_ci(NQC - 1, 0)
                epilogue_ci(NQC - 1, 1)

    _legalize_waits(nc)
    return nc


_NC_CACHE = {}


def _get_nc():
    if "nc" not in _NC_CACHE:
        _NC_CACHE["nc"] = build_nc()
    return _NC_CACHE["nc"]


def kernel(content, style, Wq, bq, Wk, bk, Wv, bv):
    content = np.asarray(content, dtype=np.float32)
    style = np.asarray(style, dtype=np.float32)
    Wq8 = _pack_pairs(np.asarray(Wq, dtype=np.float32))
    Wkb = np.ascontiguousarray(np.asarray(Wk, dtype=np.float32)).astype(NPBF16)
    Wv8 = _pack_pairs(np.asarray(Wv, dtype=np.float32))
    bqr = np.asarray(bq, dtype=np.float32).reshape(1, C)
    bkr = np.asarray(bk, dtype=np.float32).reshape(1, C)
    bvr = np.asarray(bv, dtype=np.float32).reshape(1, C).astype(NPBF16)

    nc = _get_nc()
    in_maps = []
    for core in range(8):
        b, h = core // 2, core % 2
        xt = np.ascontiguousarray(content[b].reshape(N, C).T).astype(NPBF16)
        st8 = _pack_pairs(style[b].reshape(N, C).T)
        xa = np.ascontiguousarray(xt[:, h * QH:(h + 1) * QH])
        xb = np.ascontiguousarray(xt[:, (1 - h) * QH:(2 - h) * QH])
        in_maps.append({
            "xa": xa, "xb": xb, "st": st8,
            "wq": Wq8, "wk": Wkb, "wv": Wv8,
            "bqr": bqr, "bkr": bkr, "bvr": bvr,
        })

    trace = os.environ.get("BASS_KERNEL_TRACE", "0") == "1"
    if trace:
        _install_profshim()
    res = run_bass_kernel_spmd(nc, in_maps, list(range(8)), trace=trace)
    LAST_EXEC_NS["v"] = res.exec_time_ns

    out = np.empty((B, H, W, C), dtype=np.float32)
    for core in range(8):
        b, h = core // 2, core % 2
        o = res.results[core]["out"]          # [C, QH]
        out[b].reshape(N, C)[h * QH:(h + 1) * QH, :] = o.T
    return out


# revision 16
# speedup vs baseline: 1.0371x; 1.0139x over previous
"""AdaptiveAttentionLayer on 8 TRN2 NeuronCores.

Full inputs in, full output out. Sharding: data-parallel over batch (B=4)
x 2-way sequence-parallel over the 4096 query rows -> 8 cores, each core
computes a [2048, 256] slice of one batch item's output.

The PE streams moving data at ~1 row/cycle regardless of dtype, so the
only matmul lever is fewer rows: the attention core (scores, A@V,
A@V^2 - 87% of PE work) runs as fp8e4 DoubleRow matmuls, which pack two
128-deep contractions per pass (2x). K^T is pre-normalized (1/||k||
folded in) so the softmax exp needs no per-key scale and one fused Exp
covers a 2-bank PSUM score pair. Softmax denominators: GpSimd sums each
fp8 P pair into fp16, DVE accumulates fp16 at its 4x perf mode. All
sqrt/rsqrt/reciprocal are Ln+Exp compositions so the scalar engine
keeps ONE activation table loaded (ln/exp/square/copy). PSUM plan:
score-pair 2 banks + M/E2 accumulators 4 + broadcast 1 + small rows 1.

Per-core device pipeline (channel-major / transposed layouts):
  - instance-norm stats of content/style (free-axis reductions)
  - V = style @ Wv   row-major; bias broadcast-added; V2=V*V (fp8 pairs)
  - K^T = (diag(inv_s) Wk)^T style^T + bias  channel-major bf16,
    column-l2-normalized via PE colsums + Ln/Exp + PE broadcast -> fp8
  - Q^T = Wq^T norm_content^T, l2norm likewise -> fp8
  - scores^T pair = Khat_pair qhat (fp8 DoubleRow, 512-query chunks)
  - P = exp(scores) fused per pair -> fp8
  - M^T = V^T P^T, E2^T = (V*V)^T P^T (fp8 DoubleRow, PSUM-accumulated)
  - r = sum_k P (GpSimd pair adds + DVE fp16 + PE ones-matmul),
    out = sqrt(relu(E2/r-(M/r)^2)) * norm_content + M/r
"""

import sys

if "/opt/trn_rl_repo" not in sys.path:
    sys.path.insert(0, "/opt/trn_rl_repo")

import os
import numpy as np
import ml_dtypes

import concourse.bass as bass
import concourse.mybir as mybir
import concourse.tile as tile
from concourse.bass_utils import run_bass_kernel_spmd

F32 = mybir.dt.float32
BF16 = mybir.dt.bfloat16
F16 = mybir.dt.float16
FP8 = mybir.dt.float8e4
PM = mybir.MatmulPerfMode
ALU = mybir.AluOpType
ACTF = mybir.ActivationFunctionType

B, H, W, C = 4, 64, 64, 256
N = H * W          # 4096 key/query rows per batch item
QH = N // 2        # 2048 query rows per core
NK = N // 128      # 32 key tiles
NPR = NK // 2      # 16 key-tile pairs (fp8 DoubleRow)
QC = 512           # query chunk (matmul moving free dim)
NQC = QH // QC     # 4 query chunks per core
EPS_IN = 1e-5      # instance norm eps
EPS_L2 = 1e-12     # l2norm eps
EPS_LN = 1e-30     # guards Ln(0) in sqrt-by-Ln/Exp

LAST_EXEC_NS = {"v": None}

NPBF16 = ml_dtypes.bfloat16
NPFP8 = mybir.dt.np(FP8)


def _pack_pairs(a):
    """[256, F] -> [128, 2*F] fp8 pair layout (dim1 = which 128-half)."""
    f = a.shape[1]
    return np.ascontiguousarray(
        a.reshape(2, 128, f).transpose(1, 0, 2).reshape(128, 2 * f)
    ).astype(NPFP8)


def _legalize_waits(nc):
    """This walrus build accepts at most ONE sync wait per instruction
    ('Too many sync wait commands'). Hoist extra waits onto same-engine
    NOPs inserted immediately before the offending instruction."""
    fn = nc.m.functions[0]
    nfix = 0
    for bb in fn.blocks:
        i = 0
        while i < len(bb.instructions):
            inst = bb.instructions[i]
            si = inst.sync_info
            if si is not None and len(si.on_wait) > 1:
                waits = list(si.on_wait)
                for j, w in enumerate(waits[:-1]):
                    nop = mybir.InstNoOp(
                        name=nc.get_next_instruction_name(), ins=[], outs=[]
                    )
                    nop.engine = inst.engine
                    nop.sync_info = mybir.SyncInfo(on_wait=[w], on_update=[])
                    nc.register_instruction(nop)
                    bb.instructions.insert(i + j, nop)
                i += len(waits) - 1
                inst.sync_info = mybir.SyncInfo(
                    on_wait=[waits[-1]], on_update=list(si.on_update)
                )
                nfix += 1
            i += 1
    return nfix


def _install_profshim():
    """antenv.axon_hooks is absent in this image; provide it (ctypes into
    libaxon_pjrt.so) plus an offline-safe upload_artifacts so trace=True
    yields exec_time_ns."""
    import contextlib, ctypes, types

    if "antenv.axon_hooks" in sys.modules:
        return
    so = "/opt/axon/libaxon_pjrt.so"
    hook = None
    if os.path.exists(so):
        lib = ctypes.CDLL(so)
        if hasattr(lib, "axon_start_nrt_profile"):
            lib.axon_start_nrt_profile.argtypes = [
                ctypes.POINTER(ctypes.c_int64),
                ctypes.c_size_t,
            ]
            lib.axon_start_nrt_profile.restype = ctypes.c_int64
            lib.axon_stop_nrt_profile.argtypes = [ctypes.c_char_p]
            lib.axon_stop_nrt_profile.restype = ctypes.c_int64

            @contextlib.contextmanager
            def _hook(output_dir, device_ids):
                import jax

                jax.devices()
                if device_ids:
                    ids = (ctypes.c_int64 * len(device_ids))(*device_ids)
                    rc = lib.axon_start_nrt_profile(ids, len(device_ids))
                else:
                    rc = lib.axon_start_nrt_profile(None, 0)
                if rc != 0:
                    raise RuntimeError(f"axon_start_nrt_profile rc={rc}")
                try:
                    yield
                finally:
                    n = lib.axon_stop_nrt_profile(str(output_dir).encode())
                    print(f"profile: {n} ntff file(s) -> {output_dir}",
                          file=sys.stderr)

            hook = _hook

    mod = types.ModuleType("antenv.axon_hooks")
    mod.get_axon_ntff_profile_hook = lambda: hook
    mod.set_axon_ntff_profile_hook = lambda h: None
    sys.modules["antenv.axon_hooks"] = mod

    import concourse.bass_utils as bu

    bu.upload_artifacts = lambda tmpdir: tmpdir


def build_nc():
    nc = bass.Bass()

    xa_e = nc.declare_dram_parameter("xa", [C, QH], BF16, isOutput=False)
    xb_e = nc.declare_dram_parameter("xb", [C, QH], BF16, isOutput=False)
    st_e = nc.declare_dram_parameter("st", [128, 2 * N], FP8, isOutput=False)
    wq_e = nc.declare_dram_parameter("wq", [128, 2 * C], FP8, isOutput=False)
    wk_e = nc.declare_dram_parameter("wk", [C, C], BF16, isOutput=False)
    wv_e = nc.declare_dram_parameter("wv", [128, 2 * C], FP8, isOutput=False)
    bqr_e = nc.declare_dram_parameter("bqr", [C, 1], F32, isOutput=False)
    bkr_e = nc.declare_dram_parameter("bkr", [C, 1], F32, isOutput=False)
    bvr_e = nc.declare_dram_parameter("bvr", [1, C], BF16, isOutput=False)
    out_e = nc.declare_dram_parameter("out", [C, QH], F32, isOutput=True)

    NCH_K = N // QC       # 8 key chunks
    DCH = 1024
    SCH = 2048            # stats chunk

    with tile.TileContext(nc) as tc, \
            nc.allow_low_precision(reason="fp8/bf16 attention pipeline"):
        with tc.tile_pool(name="persist", bufs=1) as pp:
            ones_col = pp.tile([128, 1], BF16)  # colsum stationary
            ones_c16 = pp.tile([128, 1], F16)   # denom colsum stationary
            ones_rbf = pp.tile([1, 128], BF16)  # bv broadcast stationary
            ones_r16 = pp.tile([1, 128], F16)   # rinv/iqr/invk broadcast
            eps_in_t = pp.tile([128, 1], F32)
            eps_l2_t = pp.tile([128, 1], F32)
            eps_ln_t = pp.tile([128, 1], F32)
            wq8 = pp.tile([128, 2, C], FP8)
            wk_s = [pp.tile([128, C], BF16, name=f"wk{i}") for i in range(2)]
            wk8 = pp.tile([128, 2, C], FP8)
            wv8 = pp.tile([128, 2, C], FP8)
            nct8 = pp.tile([128, 2, QH], FP8)
            bqc = [pp.tile([128, 1], F32, name=f"bqc{i}") for i in range(2)]
            bkc = [pp.tile([128, 1], F32, name=f"bkc{i}") for i in range(2)]
            bkc_f = [pp.tile([128, 1], F32, name=f"bkf{i}") for i in range(2)]
            bv_row = pp.tile([1, C], BF16)
            bvb = pp.tile([128, C], F32)
            # DoubleRow pair layouts (dim1 = which half of the 256-deep
            # contraction):
            #   knt8[:, co, k]      Khat^T chans co*128.., key k
            #   qnt8[:, co, q]      Qhat^T chans co*128..
            #   v8[:, pr, w, c]     V[key tile 2pr+w, chan c]
            knt_bf = pp.tile([128, 2, N], BF16)
            knt8 = pp.tile([128, 2, N], FP8)
            qnt8 = pp.tile([128, 2, QH], FP8)
            qnt = [pp.tile([128, QH], BF16, name=f"qnt{i}") for i in range(2)]
            nct = [pp.tile([128, QH], BF16, name=f"nct{i}") for i in range(2)]
            v8 = pp.tile([128, NPR, 2, C], FP8)
            v28 = pp.tile([128, NPR, 2, C], FP8)
            mean_s = [pp.tile([128, 1], F32, name=f"ms{i}") for i in range(2)]
            inv_s = [pp.tile([128, 1], F32, name=f"is{i}") for i in range(2)]
            mean_x = [pp.tile([128, 1], F32, name=f"mx{i}") for i in range(2)]
            inv_x = [pp.tile([128, 1], F32, name=f"ix{i}") for i in range(2)]

            nc.vector.memset(ones_col[:], 1.0)
            nc.vector.memset(ones_c16[:], 1.0)
            nc.vector.memset(ones_rbf[:], 1.0)
            nc.vector.memset(ones_r16[:], 1.0)
            nc.vector.memset(eps_in_t[:], EPS_IN)
            nc.vector.memset(eps_l2_t[:], EPS_L2)
            nc.vector.memset(eps_ln_t[:], EPS_LN)

            # ================= phase 1: stats + projections =================
            with (
                tc.tile_pool(name="inputs", bufs=1) as tp,
                tc.tile_pool(name="w1", bufs=2) as w1,
                tc.tile_pool(name="psum1", bufs=3, space="PSUM") as ps1,
            ):
                st8 = tp.tile([128, 2, N], FP8, name="st8")
                xa_t = [tp.tile([128, QH], BF16, name=f"xa{i}")
                        for i in range(2)]
                for w in range(2):
                    nc.sync.dma_start(wv8[:, w, :], wv_e[:, w * C:(w + 1) * C])
                    nc.sync.dma_start(wq8[:, w, :], wq_e[:, w * C:(w + 1) * C])
                for i in range(2):
                    nc.sync.dma_start(wk_s[i][:],
                                      wk_e[i * 128:(i + 1) * 128, :])
                    nc.sync.dma_start(bqc[i][:], bqr_e[i * 128:(i + 1) * 128, :])
                    nc.sync.dma_start(bkc[i][:], bkr_e[i * 128:(i + 1) * 128, :])
                nc.sync.dma_start(bv_row[:], bvr_e[:])
                for j in range(0, N, DCH):
                    for i in range(2):
                        nc.sync.dma_start(
                            st8[:, i, j:j + DCH],
                            st_e[:, i * N + j:i * N + j + DCH])
                for j in range(0, QH, DCH):
                    for i in range(2):
                        nc.sync.dma_start(
                            xa_t[i][:, j:j + DCH],
                            xa_e[i * 128:(i + 1) * 128, j:j + DCH],
                        )

                # bv broadcast for V row-major bias add
                ps_bc = ps1.tile([128, C], F32, name="ps_bc", tag="prj")
                nc.tensor.matmul(ps_bc[:], ones_rbf[:], bv_row[:])
                nc.vector.tensor_copy(bvb[:], ps_bc[:])

                def stats_closures(chunks, mean, inv, i):
                    """Return a list of closures; call them in order, spaced
                    between PE-heavy work. Last closure finalizes stats."""
                    nck = len(chunks)
                    parts = w1.tile([128, nck], F32, name="parts",
                                    bufs=2)
                    parts_m = w1.tile([128, nck], F16, name="parts_m",
                                      bufs=2)
                    out = []

                    def chunk_op(j, ch):
                        def go():
                            scr = w1.tile([128, N], BF16, name="sqscr",
                                          bufs=2)
                            nc.scalar.activation(
                                scr[:, 0:ch.free_size()], ch, ACTF.Square,
                                accum_out=parts[:, j:j + 1],
                            )
                            nc.vector.tensor_reduce(
                                parts_m[:, j:j + 1], ch,
                                axis=mybir.AxisListType.X, op=ALU.add,
                            )
                        return go

                    for j, ch in enumerate(chunks):
                        out.append(chunk_op(j, ch))

                    def finalize():
                        ssq = w1.tile([128, 1], F32, name="ssq")
                        nc.vector.reduce_sum(ssq[:], parts[:, 0:nck],
                                             axis=mybir.AxisListType.X)
                        ssum = w1.tile([128, 1], F32, name="ssum")
                        nc.vector.reduce_sum(ssum[:], parts_m[:, 0:nck],
                                             axis=mybir.AxisListType.X)
                        nc.vector.tensor_scalar_mul(mean[i][:], ssum[:],
                                                    1.0 / N)
                        ex2 = w1.tile([128, 1], F32, name="ex2")
                        nc.vector.tensor_scalar_mul(ex2[:], ssq[:], 1.0 / N)
                        msq = w1.tile([128, 1], F32, name="msq")
                        nc.vector.tensor_mul(msq[:], mean[i][:], mean[i][:])
                        var = w1.tile([128, 1], F32, name="var")
                        nc.vector.tensor_sub(var[:], ex2[:], msq[:])
                        # 1/sqrt(var+eps) = Exp(-0.5*Ln(var+eps))
                        lnv = w1.tile([128, 1], F32, name="lnv")
                        nc.scalar.activation(lnv[:], var[:], ACTF.Ln,
                                             bias=eps_in_t[:])
                        nc.scalar.activation(inv[i][:], lnv[:], ACTF.Exp,
                                             scale=-0.5)
                    out.append(finalize)
                    return out

                style_ops = []
                for i in range(2):
                    style_ops += stats_closures([st8[:, i, :]],
                                                mean_s, inv_s, i)

                # ---- V projection (row-major; bias added at evacuation
                # straight into the fp8 pair layout); V2 = V*V behind it.
                # style-stats ops interleaved so they don't head-of-line
                # block the V PSUM evacuations
                for kt in range(NK):
                    ksl = slice(kt * 128, (kt + 1) * 128)
                    ps_v = ps1.tile([128, C], F32, name="ps_v", tag="prj")
                    nc.tensor.matmul(ps_v[:], st8[:, :, ksl], wv8[:],
                                     start=True, stop=True,
                                     perf_mode=PM.DoubleRow)
                    vdst = v8[:, kt // 2, kt % 2, :]
                    nc.vector.tensor_add(vdst, ps_v[:], bvb[:])
                    nc.gpsimd.tensor_mul(v28[:, kt // 2, kt % 2, :],
                                         vdst, vdst)
                    if kt % 3 == 2 and style_ops:
                        style_ops.pop(0)()
                while style_ops:
                    style_ops.pop(0)()

                # ---- fold style instance norm into Wk; column bias corr
                for i in range(2):
                    nc.vector.tensor_scalar_mul(wk_s[i][:], wk_s[i][:],
                                                inv_s[i][:])
                mu_inv = [w1.tile([128, 1], BF16, name=f"mi{i}")
                          for i in range(2)]
                for i in range(2):
                    nc.vector.tensor_mul(mu_inv[i][:], mean_s[i][:],
                                         inv_s[i][:])
                for co in range(2):
                    ps_c = ps1.tile([128, 1], F32, name="ps_c", tag="pn",
                                    bufs=2)
                    csl = slice(co * 128, (co + 1) * 128)
                    nc.tensor.matmul(ps_c[:], wk_s[0][:, csl],
                                     mu_inv[0][:], start=True, stop=False)
                    nc.tensor.matmul(ps_c[:], wk_s[1][:, csl],
                                     mu_inv[1][:], start=False, stop=True)
                    nc.vector.tensor_sub(bkc_f[co][:], bkc[co][:], ps_c[:])
                for w in range(2):
                    nc.vector.tensor_copy(wk8[:, w, :], wk_s[w][:])

                # ---- K^T projection (channel-major bf16) + column
                # sumsq + per-chunk l2 normalization into fp8 (pipelined
                # so the norm chain hides under later chunks' matmuls)

                def proj_t(src, w_t, bias_c, nch, interleave=None):
                    def colsum(ch, sq):
                        csl = slice(ch * QC, (ch + 1) * QC)
                        ps_n = ps1.tile([1, QC], F32, name="ps_n", tag="pn",
                                        bufs=2)
                        nc.tensor.matmul(ps_n[:], ones_col[:],
                                         sq[0][:], start=True, stop=False)
                        nc.tensor.matmul(ps_n[:], ones_col[:],
                                         sq[1][:], start=False, stop=True)
                        lnk = w1.tile([1, QC], F32, name="lnk", bufs=2)
                        nc.scalar.activation(lnk[:], ps_n[:], ACTF.Ln,
                                             bias=eps_l2_t[0:1, :])
                        ivr = w1.tile([1, QC], F16, name="ivr", bufs=2)
                        nc.scalar.activation(ivr[:], lnk[:], ACTF.Exp,
                                             scale=-0.5)
                        ps_b = ps1.tile([128, QC], F32, name="ps_b",
                                        tag="pbig")
                        nc.tensor.matmul(ps_b[:], ones_r16[:], ivr[:])
                        ivb = w1.tile([128, QC], BF16, name="ivb", bufs=2)
                        nc.scalar.activation(ivb[:], ps_b[:], ACTF.Copy)
                        for co in range(2):
                            nc.gpsimd.tensor_mul(knt8[:, co, csl],
                                                 knt_bf[:, co, csl], ivb[:])

                    pend = None
                    for ch in range(nch):
                        csl = slice(ch * QC, (ch + 1) * QC)
                        sq = []
                        for co in range(2):
                            wsl = slice(co * 128, (co + 1) * 128)
                            ps_p = ps1.tile([128, QC], F32, name="ps_p",
                                            tag="pbig")
                            nc.tensor.matmul(ps_p[:], w_t[:, :, wsl],
                                             src[:, :, csl],
                                             start=True, stop=True,
                                             perf_mode=PM.DoubleRow)
                            kdst = knt_bf[:, co, csl]
                            nc.vector.tensor_scalar(
                                out=kdst, in0=ps_p[:],
                                scalar1=bias_c[co][:], scalar2=None,
                                op0=ALU.add)
                            s = w1.tile([128, QC], BF16, name="sqc", bufs=3)
                            nc.scalar.activation(s[:], ps_p[:], ACTF.Square,
                                                 bias=bias_c[co][:])
                            sq.append(s)
                        if pend is not None:
                            colsum(*pend)
                        pend = (ch, sq)
                        if interleave:
                            interleave.pop(0)()
                    colsum(*pend)
                    while interleave:
                        interleave.pop(0)()

                # content stats + norm_content interleaved into the K
                # projection (they wait on the later xa/xb DMA; queuing
                # them first would stall the K chain's engines)
                xbch = {}
                for i in range(2):
                    cb = tp.tile([128, QH], BF16, name="xbs", bufs=2)
                    nc.sync.dma_start(cb[:], xb_e[i * 128:(i + 1) * 128, :])
                    xbch[i] = cb
                content_ops = []
                for i in range(2):
                    content_ops += stats_closures([xa_t[i][:], xbch[i][:]],
                                                  mean_x, inv_x, i)

                def nct_op(i):
                    def go():
                        nc.vector.tensor_scalar(
                            out=nct[i][:], in0=xa_t[i][:],
                            scalar1=mean_x[i][:], scalar2=inv_x[i][:],
                            op0=ALU.subtract, op1=ALU.mult,
                        )
                        nc.vector.tensor_copy(nct8[:, i, :], nct[i][:])
                    return go

                content_ops += [nct_op(0), nct_op(1)]
                proj_t(st8, wk8, bkc_f, NCH_K, content_ops)


            # ========== phase 2: attention ==========
            with (
                tc.tile_pool(name="w2", bufs=2) as w2,
                tc.tile_pool(name="psum_acc", bufs=1, space="PSUM") as psa,
                tc.tile_pool(name="psum_pair", bufs=1, space="PSUM") as ppx,
                tc.tile_pool(name="psum_small", bufs=1, space="PSUM") as psl,
            ):
                state = {}
                qstate = {}

                def qproj_a(qc):
                    """Project Q chunk qc into qnt (channel-major) and
                    square for column norms."""
                    csl = slice(qc * QC, (qc + 1) * QC)
                    pq = ppx.tile([128, 2 * QC], F32, name="pair")
                    for co in range(2):
                        wsl = slice(co * 128, (co + 1) * 128)
                        osl = slice(co * QC, (co + 1) * QC)
                        nc.tensor.matmul(pq[:, osl], wq8[:, :, wsl],
                                         nct8[:, :, csl],
                                         start=True, stop=True,
                                         perf_mode=PM.DoubleRow)
                    sq = []
                    for co in range(2):
                        osl = slice(co * QC, (co + 1) * QC)
                        nc.vector.tensor_scalar(
                            out=qnt[co][:, csl], in0=pq[:, osl],
                            scalar1=bqc[co][:], scalar2=None, op0=ALU.add)
                        s = w2.tile([128, QC], BF16, name="qsq", bufs=2)
                        nc.scalar.activation(s[:], qnt[co][:, csl],
                                             ACTF.Square)
                        sq.append(s)
                    qstate[qc] = sq

                def qproj_b(qc):
                    """Column sumsq -> 1/norm row for chunk qc."""
                    sq = qstate.pop(qc)
                    ps_n = psl.tile([1, QC], F32, name="ps_n", tag="small")
                    nc.tensor.matmul(ps_n[:], ones_col[:], sq[0][:],
                                     start=True, stop=False)
                    nc.tensor.matmul(ps_n[:], ones_col[:], sq[1][:],
                                     start=False, stop=True)
                    lnq = w2.tile([1, QC], F32, name="lnq", bufs=1)
                    nc.scalar.activation(lnq[:], ps_n[:], ACTF.Ln,
                                         bias=eps_l2_t[0:1, :])
                    iqr = w2.tile([1, QC], F16, name="invr", bufs=2)
                    nc.scalar.activation(iqr[:], lnq[:], ACTF.Exp,
                                         scale=-0.5)
                    qstate[qc] = iqr

                def qproj_c(qc):
                    """Broadcast 1/norm and scale Q chunk qc columns."""
                    iqr = qstate.pop(qc)
                    csl = slice(qc * QC, (qc + 1) * QC)
                    ps_b = psa.tile([128, QC], F32, name="qps_b",
                                    tag="ps_rb")
                    nc.tensor.matmul(ps_b[:], ones_r16[:], iqr[:])
                    for co in range(2):
                        nc.vector.tensor_mul(qnt[co][:, csl],
                                             qnt[co][:, csl], ps_b[:])

                def qproj_d(qc):
                    """fp8 copy of the scaled Q chunk (DoubleRow moving)."""
                    csl = slice(qc * QC, (qc + 1) * QC)
                    for co in range(2):
                        nc.vector.tensor_copy(qnt8[:, co, csl],
                                              qnt[co][:, csl])

                qproj_a(0)
                qproj_b(0)
                qproj_c(0)
                qproj_d(0)

                def denom(qc):
                    """Softmax denominator for chunk qc (emitted a few
                    iterations into chunk qc+1)."""
                    racc, msb, esb = state[qc]
                    ps_r = psl.tile([1, QC], F32, name="ps_r", tag="small")
                    nc.tensor.matmul(ps_r[:], ones_c16[:], racc[:])
                    lnr = w2.tile([1, QC], F32, name="lnr", bufs=1)
                    nc.scalar.activation(lnr[:], ps_r[:], ACTF.Ln)
                    rinv_row = w2.tile([1, QC], F16, name="rinv_row",
                                       bufs=1)
                    nc.scalar.activation(rinv_row[:], lnr[:], ACTF.Exp,
                                         scale=-1.0)
                    ps_rb = psa.tile([128, QC], F32, name="ps_rb")
                    nc.tensor.matmul(ps_rb[:], ones_r16[:], rinv_row[:])
                    rinv = w2.tile([128, QC], F16, name="rinv", bufs=2)
                    nc.vector.tensor_copy(rinv[:], ps_rb[:])
                    state[qc] = (racc, msb, esb, rinv)

                def epilogue_ci(qc, ci):
                    _, msb, esb, rinv = state[qc]
                    qsl = slice(qc * QC, (qc + 1) * QC)
                    mhat = w2.tile([128, QC], F16, name="mhat", bufs=2)
                    nc.vector.tensor_mul(mhat[:], msb[ci][:], rinv[:])
                    ehat = w2.tile([128, QC], F16, name="ehat", bufs=2)
                    nc.vector.tensor_mul(ehat[:], esb[ci][:], rinv[:])
                    s2p = w2.tile([128, QC], F16, name="s2p", bufs=2)
                    nc.scalar.activation(s2p[:], mhat[:], ACTF.Square)
                    s2 = w2.tile([128, QC], F16, name="s2", bufs=2)
                    nc.vector.tensor_sub(s2[:], ehat[:], s2p[:])
                    nc.vector.tensor_scalar_max(s2[:], s2[:], 0.0)
                    # sqrt(s2) = Exp(0.5*Ln(s2+tiny)); ln stays fp32 (its
                    # absolute error is amplified by the exp)
                    ln2 = w2.tile([128, QC], F32, name="ln2", bufs=2)
                    nc.scalar.activation(ln2[:], s2[:], ACTF.Ln,
                                         bias=eps_ln_t[:])
                    s_sb = w2.tile([128, QC], F16, name="s_sb", bufs=2)
                    nc.scalar.activation(s_sb[:], ln2[:], ACTF.Exp,
                                         scale=0.5)
                    o_sb = w2.tile([128, QC], F16, name="o_sb", bufs=2)
                    nc.vector.tensor_mul(o_sb[:], s_sb[:], nct[ci][:, qsl])
                    o_f = w2.tile([128, QC], F32, name="o_f", bufs=2)
                    nc.vector.tensor_add(o_f[:], o_sb[:], mhat[:])
                    nc.sync.dma_start(
                        out_e[ci * 128:(ci + 1) * 128, qsl], o_f[:]
                    )
                    if ci == 1:
                        state.pop(qc)

                for qc in range(NQC):
                    qsl = slice(qc * QC, (qc + 1) * QC)
                    ps_m = [psa.tile([128, QC], F32, name=f"ps_m{c}")
                            for c in range(2)]
                    ps_e = [psa.tile([128, QC], F32, name=f"ps_e{c}")
                            for c in range(2)]
                    racc = w2.tile([128, QC], F16, name="racc")

                    def emit_av(pr, p8):
                        first, last = pr == 0, pr == NPR - 1
                        for ci in range(2):
                            cs = slice(ci * 128, (ci + 1) * 128)
                            nc.tensor.matmul(ps_m[ci][:],
                                             v8[:, pr, :, cs], p8[:],
                                             start=first, stop=last,
                                             perf_mode=PM.DoubleRow)
                            nc.tensor.matmul(ps_e[ci][:],
                                             v28[:, pr, :, cs], p8[:],
                                             start=first, stop=last,
                                             perf_mode=PM.DoubleRow)

                    pend = None
                    for pr in range(NPR):
                        pair = ppx.tile([128, 2 * QC], F32, name="pair")
                        for wh in range(2):
                            kt = 2 * pr + wh
                            ksl = slice(kt * 128, (kt + 1) * 128)
                            nc.tensor.matmul(pair[:, wh * QC:(wh + 1) * QC],
                                             knt8[:, :, ksl],
                                             qnt8[:, :, qsl],
                                             start=True, stop=True,
                                             perf_mode=PM.DoubleRow)
                        p8 = w2.tile([128, 2, QC], FP8, name="p8", bufs=4)
                        nc.scalar.activation(p8[:], pair[:], ACTF.Exp)
                        padd = w2.tile([128, QC], F16, name="padd", bufs=3)
                        nc.gpsimd.tensor_add(padd[:], p8[:, 0, :],
                                             p8[:, 1, :])
                        if pr == 0:
                            nc.vector.tensor_copy(racc[:], padd[:])
                        else:
                            nc.vector.tensor_add(racc[:], racc[:], padd[:])
                        if qc > 0:
                            if pr == 1:
                                denom(qc - 1)
                            elif pr == 3:
                                epilogue_ci(qc - 1, 0)
                            elif pr == 5:
                                epilogue_ci(qc - 1, 1)
                        if qc + 1 < NQC:
                            if pr == 8:
                                qproj_a(qc + 1)
                            elif pr == 10:
                                qproj_b(qc + 1)
                            elif pr == 12:
                                qproj_c(qc + 1)
                            elif pr == 13:
                                qproj_d(qc + 1)
                        if pend is not None:
                            emit_av(*pend)
                        pend = (pr, p8)
                        if qc == NQC - 1 and pr == NPR - 1:
                            state[qc] = (racc, None, None)
                            denom(qc)
                            dstate = state.pop(qc)
                    emit_av(*pend)
                    # evacuate accumulators fast (ACT) to free PSUM banks
                    msb = [w2.tile([128, QC], F16, name=f"msb{c}")
                           for c in range(2)]
                    esb = [w2.tile([128, QC], F16, name=f"esb{c}")
                           for c in range(2)]
                    for ci in range(2):
                        nc.scalar.activation(msb[ci][:], ps_m[ci][:],
                                             ACTF.Copy)
                        nc.scalar.activation(esb[ci][:], ps_e[ci][:],
                                             ACTF.Copy)
                    if qc == NQC - 1:
                        state[qc] = (dstate[0], msb, esb, dstate[3])
                    else:
                        state[qc] = (racc, msb, esb)
                epilogue_ci(NQC - 1, 0)
                epilogue_ci(NQC - 1, 1)

    _legalize_waits(nc)
    return nc


_NC_CACHE = {}


def _get_nc():
    if "nc" not in _NC_CACHE:
        _NC_CACHE["nc"] = build_nc()
    return _NC_CACHE["nc"]


def kernel(content, style, Wq, bq, Wk, bk, Wv, bv):
    content = np.asarray(content, dtype=np.float32)
    style = np.asarray(style, dtype=np.float32)
    Wq8 = _pack_pairs(np.asarray(Wq, dtype=np.float32))
    Wkb = np.ascontiguousarray(np.asarray(Wk, dtype=np.float32)).astype(NPBF16)
    Wv8 = _pack_pairs(np.asarray(Wv, dtype=np.float32))
    bqr = np.asarray(bq, dtype=np.float32).reshape(1, C)
    bkr = np.asarray(bk, dtype=np.float32).reshape(1, C)
    bvr = np.asarray(bv, dtype=np.float32).reshape(1, C).astype(NPBF16)

    nc = _get_nc()
    in_maps = []
    for core in range(8):
        b, h = core // 2, core % 2
        xt = np.ascontiguousarray(content[b].reshape(N, C).T).astype(NPBF16)
        st8 = _pack_pairs(style[b].reshape(N, C).T)
        xa = np.ascontiguousarray(xt[:, h * QH:(h + 1) * QH])
        xb = np.ascontiguousarray(xt[:, (1 - h) * QH:(2 - h) * QH])
        in_maps.append({
            "xa": xa, "xb": xb, "st": st8,
            "wq": Wq8, "wk": Wkb, "wv": Wv8,
            "bqr": bqr, "bkr": bkr, "bvr": bvr,
        })

    trace = os.environ.get("BASS_KERNEL_TRACE", "0") == "1"
    if trace:
        _install_profshim()
    res = run_bass_kernel_spmd(nc, in_maps, list(range(8)), trace=trace)
    LAST_EXEC_NS["v"] = res.exec_time_ns

    out = np.empty((B, H, W, C), dtype=np.float32)
    for core in range(8):
        b, h = core // 2, core % 2
        o = res.results[core]["out"]          # [C, QH]
        out[b].reshape(N, C)[h * QH:(h + 1) * QH, :] = o.T
    return out
